# revision 1
# baseline (speedup 1.0000x reference)
"""DilatedRnnStack kernel for 8 TRN2 NeuronCores (Bass/Tile, SPMD).

Strategy
--------
Tensor-parallel split of the state dim S=1024: core j owns s-rows
[128j, 128j+128) of every gate of all 4 layers. Everything on-chip is
kept feature-major ([s-dim on partitions, batch on free]) so weights are
the matmul stationary operand (fp16 -> fast weight load) and no
transposes are needed anywhere; the host pre-transposes x / W / Wa into
the exact SBUF images. After each cell the cores AllGather their
[128,64] slice of `whole` (fp16, moved as fp32-typed bytes) into a
replicated [1024,64] buffer that directly feeds later cells' matmuls as
[128,64] rhs chunks. Cells are emitted in wavefront order (cell (l,t) in
wave t+l) so up to 4 independent cells pipeline across PE/ACT/DVE/DMA
and the collectives hide under compute. The final adaptor matmul is done
once at the end over all T timesteps (batched moving dim), each core
producing its own 64-wide slice of y^T; the host reassembles.
"""
import sys

sys.path.insert(0, "/opt/trn_rl_repo")
import numpy as np

DILS = (1, 2, 4, 8)
T, B = 16, 64
H, S = 256, 1024
DIN, DOUT = 512, 512
NC = 8
DEPTH = tuple(d + 1 for d in DILS)
# gate ids: 0=f, 1=n, 2=a, 3=o; layer 0 (d=1) never uses 'a'
MTILES = ((0, 1, 3), (0, 1, 2, 3), (0, 1, 2, 3), (0, 1, 2, 3))
KIN = (8, 10, 10, 10)  # K chunks of 128 including prevH/dH
NU = (4, 6, 6, 6)  # u chunks

_CACHED_NC = None


def _build_kernel():
    import concourse.bacc as bacc
    import concourse.tile as tile
    from concourse import mybir

    f32 = mybir.dt.float32
    mdt = mybir.dt.float16
    AF = mybir.ActivationFunctionType

    nc = bacc.Bacc(
        "TRN2", target_bir_lowering=False, debug=False, num_devices=NC
    )

    xt_d = nc.dram_tensor("xt", [128, T, 4, B], mdt, kind="ExternalInput")
    w_d = [
        nc.dram_tensor(
            f"w{l}", [128, KIN[l], 128 * len(MTILES[l])], mdt, kind="ExternalInput"
        )
        for l in range(4)
    ]
    bias_d = nc.dram_tensor("bias", [128, 16], f32, kind="ExternalInput")
    wat_d = nc.dram_tensor("wat", [128, 6, 64], mdt, kind="ExternalInput")
    bay_d = nc.dram_tensor("bay", [64, 1], f32, kind="ExternalInput")
    y_d = nc.dram_tensor("y", [64, T * B], f32, kind="ExternalOutput")

    with tile.TileContext(nc) as tc:
        with (
            tc.tile_pool(name="const", bufs=1) as const,
            tc.tile_pool(name="work", bufs=4) as work,
            tc.tile_pool(name="psum", bufs=4, space="PSUM") as psum,
            tc.tile_pool(name="ypsum", bufs=2, space="PSUM") as ypsum,
            tc.tile_pool(name="dram", bufs=16, space="DRAM") as dram,
        ):
            xsb = const.tile([128, T, 4, B], mdt, tag="xsb")
            nc.sync.dma_start(xsb[:], xt_d[:])
            wsb = []
            for l in range(4):
                t_ = const.tile(
                    [128, KIN[l], 128 * len(MTILES[l])], mdt, tag=f"w{l}"
                )
                nc.sync.dma_start(t_[:], w_d[l][:])
                wsb.append(t_)
            bsb = const.tile([128, 16], f32, tag="bias")
            nc.sync.dma_start(bsb[:], bias_d[:])
            watsb = const.tile([128, 6, 64], mdt, tag="wat")
            nc.sync.dma_start(watsb[:], wat_d[:])
            baysb = const.tile([64, 1], f32, tag="bay")
            nc.sync.dma_start(baysb[:], bay_d[:])

            whole = [
                [
                    const.tile([128, 8, B], mdt, tag=f"wh{l}_{i}", name=f"wh{l}_{i}")
                    for i in range(DEPTH[l])
                ]
                for l in range(4)
            ]
            cring = [
                [
                    const.tile([128, B], f32, tag=f"c{l}_{i}", name=f"c{l}_{i}")
                    for i in range(DEPTH[l])
                ]
                for l in range(4)
            ]
            blockBuf = const.tile([128, 6, T * B], mdt, tag="blockBuf")
            ysb = const.tile([64, T * B], f32, tag="ysb")

            def bias_ap(l, mi):
                return bsb[:, 4 * l + mi : 4 * l + mi + 1]

            def emit_cell(l, t):
                d = DILS[l]
                gates = MTILES[l]
                chunks = []
                if l == 0:
                    for c in range(4):
                        chunks.append((c, xsb[:, t, c, :]))
                else:
                    wprev = whole[l - 1][t % DEPTH[l - 1]]
                    for c in range(6):
                        chunks.append((c, wprev[:, c, :]))
                if t >= 1:
                    nu = NU[l]
                    wh_tm1 = whole[l][(t - 1) % DEPTH[l]]
                    for i in range(2):
                        chunks.append((nu + i, wh_tm1[:, 6 + i, :]))
                    src_d = whole[l][(t - d) % DEPTH[l]] if t - d >= 0 else wh_tm1
                    for i in range(2):
                        chunks.append((nu + 2 + i, src_d[:, 6 + i, :]))
                if t == 0:
                    act_m = [mi for mi, g in enumerate(gates) if g in (1, 3)]
                else:
                    act_m = list(range(len(gates)))
                g_ps = psum.tile([128, 4 * B], f32, tag="g")
                for mi in act_m:
                    for ci, (k, rhs) in enumerate(chunks):
                        nc.tensor.matmul(
                            g_ps[:, B * mi : B * (mi + 1)],
                            wsb[l][:, k, 128 * mi : 128 * (mi + 1)],
                            rhs,
                            start=(ci == 0),
                            stop=(ci == len(chunks) - 1),
                        )

                def mslot(gid):
                    return gates.index(gid)

                def gp(gid):
                    mi = mslot(gid)
                    return g_ps[:, B * mi : B * (mi + 1)]

                stage = wave_stage[0][:, l, :]
                cnew = cring[l][t % DEPTH[l]]
                if t == 0:
                    o_t = work.tile([128, B], f32, tag="o")
                    nc.scalar.activation(
                        cnew[:], gp(1), AF.Tanh, bias=bias_ap(l, mslot(1))
                    )
                    nc.scalar.activation(
                        o_t[:], gp(3), AF.Sigmoid, bias=bias_ap(l, mslot(3))
                    )
                    nc.vector.tensor_mul(stage[:], o_t[:], cnew[:])
                else:
                    f_t = work.tile([128, B], f32, tag="f")
                    n_t = work.tile([128, B], f32, tag="n")
                    o_t = work.tile([128, B], f32, tag="o")
                    nc.scalar.activation(
                        f_t[:], gp(0), AF.Sigmoid, bias=bias_ap(l, mslot(0))
                    )
                    nc.scalar.activation(
                        n_t[:], gp(1), AF.Tanh, bias=bias_ap(l, mslot(1))
                    )
                    nc.scalar.activation(
                        o_t[:], gp(3), AF.Sigmoid, bias=bias_ap(l, mslot(3))
                    )
                    prevC = cring[l][(t - 1) % DEPTH[l]]
                    if d > 1 and t - d >= 0:
                        a_t = work.tile([128, B], f32, tag="a")
                        t1 = work.tile([128, B], f32, tag="t1")
                        nc.scalar.activation(
                            a_t[:], gp(2), AF.Sigmoid, bias=bias_ap(l, mslot(2))
                        )
                        dC = cring[l][(t - d) % DEPTH[l]]
                        nc.vector.tensor_sub(t1[:], prevC[:], dC[:])
                        nc.vector.tensor_mul(t1[:], a_t[:], t1[:])
                        nc.vector.tensor_add(t1[:], t1[:], dC[:])
                        wC = t1
                    else:
                        wC = prevC
                    t2 = work.tile([128, B], f32, tag="t2")
                    nc.vector.tensor_sub(t2[:], wC[:], n_t[:])
                    nc.vector.tensor_mul(t2[:], f_t[:], t2[:])
                    nc.vector.tensor_add(cnew[:], t2[:], n_t[:])
                    nc.vector.tensor_mul(stage[:], o_t[:], cnew[:])

            # one AllGather per wavefront: the wave's (up to) 4 cells stage
            # their fp16 slices into one tile; a single collective gathers
            # them (as fp32-typed bytes) and per-cell readbacks scatter into
            # the whole-rings.
            wave_stage = [None]
            for w in range(T + 3):
                cells = [(l, w - l) for l in range(4) if 0 <= w - l < T]
                wave_stage[0] = work.tile([128, 4, B], mdt, tag="wstage", name="wstage")
                for l, t in cells:
                    emit_cell(l, t)
                lmin = min(l for l, _ in cells)
                nv = len(cells)
                b_in = dram.tile(
                    [128, nv * (B // 2)], f32, tag="bin", name="bin", bufs=4
                )
                b_out = dram.tile(
                    [S, nv * (B // 2)], f32, addr_space="Shared",
                    tag="bout", name="bout", bufs=4,
                )
                nc.scalar.dma_start(
                    b_in[:], wave_stage[0][:, lmin : lmin + nv, :].bitcast(f32)
                )
                nc.gpsimd.collective_compute(
                    "AllGather",
                    mybir.AluOpType.bypass,
                    ins=[b_in[:].opt()],
                    outs=[b_out[:].opt()],
                    replica_groups=[list(range(NC))],
                )
                bo = b_out[:].rearrange("(j p) (l b) -> p l j b", p=128, l=nv)
                for l, t in cells:
                    wh = whole[l][t % DEPTH[l]]
                    nc.sync.dma_start(wh[:].bitcast(f32), bo[:, l - lmin, :, :])
                    if l == 1:
                        nc.vector.tensor_copy(
                            blockBuf[:, :, B * t : B * (t + 1)], wh[:, 0:6, :]
                        )
                    elif l == 3:
                        nc.vector.tensor_add(
                            blockBuf[:, :, B * t : B * (t + 1)],
                            blockBuf[:, :, B * t : B * (t + 1)],
                            wh[:, 0:6, :],
                        )

            # adaptor: yT = Wa @ (out1 + out3), batched over all T
            NTOT = T * B
            for h in range(NTOT // 512):
                yp = ypsum.tile([64, 512], f32, tag="yp")
                for j in range(6):
                    nc.tensor.matmul(
                        yp[:],
                        watsb[:, j, :],
                        blockBuf[:, j, 512 * h : 512 * (h + 1)],
                        start=(j == 0),
                        stop=(j == 5),
                    )
                nc.scalar.activation(
                    ysb[:, 512 * h : 512 * (h + 1)],
                    yp[:],
                    AF.Identity,
                    bias=baysb[:, 0:1],
                )
            nc.sync.dma_start(y_d[:], ysb[:])

    nc.compile()
    return nc


def _prep_inputs(inputs):
    mmdt = np.float16
    x = np.ascontiguousarray(inputs["x"], dtype=np.float32)
    Ws = [np.asarray(inputs[f"W{l}"], np.float32) for l in range(4)]
    bs = [np.asarray(inputs[f"b{l}"], np.float32) for l in range(4)]
    Wa = np.asarray(inputs["Wa"], np.float32)
    ba = np.asarray(inputs["ba"], np.float32)

    xt = x.transpose(2, 0, 1).reshape(4, 128, T, B).transpose(1, 2, 0, 3)
    xt = np.ascontiguousarray(xt, dtype=mmdt)

    in_maps = []
    for j in range(NC):
        m = {"xt": xt}
        for l in range(4):
            rows = np.concatenate(
                [
                    np.arange(g * S + 128 * j, g * S + 128 * (j + 1))
                    for g in MTILES[l]
                ]
            )
            Wsel = Ws[l][rows, :]
            nm = len(MTILES[l])
            wt = Wsel.T.reshape(KIN[l], 128, nm * 128).transpose(1, 0, 2)
            m[f"w{l}"] = np.ascontiguousarray(wt, dtype=mmdt)
        bias = np.zeros((128, 16), np.float32)
        for l in range(4):
            for mi, g in enumerate(MTILES[l]):
                v = bs[l][g * S + 128 * j : g * S + 128 * (j + 1)].copy()
                if g == 0:
                    v += 1.0  # fold the forget-gate +1 shift
                bias[:, 4 * l + mi] = v
        m["bias"] = bias
        wa_j = Wa[64 * j : 64 * (j + 1), :]
        wat = wa_j.T.reshape(6, 128, 64).transpose(1, 0, 2)
        m["wat"] = np.ascontiguousarray(wat, dtype=mmdt)
        m["bay"] = np.ascontiguousarray(ba[64 * j : 64 * (j + 1), None])
        in_maps.append(m)
    return in_maps


def kernel(**inputs) -> np.ndarray:
    global _CACHED_NC
    from concourse import bass_utils

    if _CACHED_NC is None:
        _CACHED_NC = _build_kernel()
    in_maps = _prep_inputs(inputs)
    res = bass_utils.run_bass_kernel_spmd(
        _CACHED_NC, in_maps, core_ids=list(range(NC))
    )
    y = np.zeros((T, B, DOUT), np.float32)
    for j in range(NC):
        yj = res.results[j]["y"].reshape(64, T, B)
        y[:, :, 64 * j : 64 * (j + 1)] = yj.transpose(1, 2, 0)
    return y

